# revision 1
# baseline (speedup 1.0000x reference)
"""Distributed Bass kernel for causal multi-head attention with RoPE.

Problem: B=2, S=2048, D=2048, H=16, HD=128 (nn_Attention_85315230368481).

Sharding: sequence-parallel over 8 cores. Core c owns query rows
[c*256, (c+1)*256) of both batches (512 rows total). Each core projects
Q/K/V for its own rows, applies RoPE to Q and K, AllGathers K^T and V
across cores (bf16), then computes full attention for its query rows
over all keys, and the output projection for its rows. The host
concatenates the 8 row-shards into the full output. No AllReduce.

Layout tricks:
 - x is passed transposed ([D, 512]) so Q^T/K^T ([head_dim, rows]) and
   V (natural [rows, D]) all come straight out of the PE array.
 - Wq/Wk columns are permuted per head (even dims then odd dims) so RoPE
   works on contiguous partition halves; scores are invariant to the
   permutation since both Q and K use it.
 - Scores are computed transposed ([keys, queries]) so exp(scores) is
   directly the moving operand of the attention*V matmul, and the
   softmax denominator is an accumulating ones-column matmul.
 - All matmuls in bf16 (inputs rounded; accumulation stays fp32 in
   PSUM), which runs the PE at full rate with fast weight loads and
   halves every DMA/collective byte count.
"""

import sys

import ml_dtypes
import numpy as np

if "/opt/trn_rl_repo" not in sys.path:
    sys.path.insert(0, "/opt/trn_rl_repo")

B, S, D, H = 2, 2048, 2048, 16
HD = D // H            # 128
NCORES = 8
SQ = S // NCORES       # 256 query rows per core per batch
ROWS = B * SQ          # 512 rows per core
DCH = D // 128         # 16 contraction chunks
SCALE = 1.0 / float(np.sqrt(HD))
BF16 = ml_dtypes.bfloat16

_GRAPH = None
_TRACE = False
_LAST_EXEC_NS = None
_LAST_RES = None


def _build_graph():
    import concourse.mybir as mybir
    from concourse import bacc, tile

    f32 = mybir.dt.float32
    bf = mybir.dt.bfloat16
    Exp = mybir.ActivationFunctionType.Exp

    nc = bacc.Bacc("TRN2", target_bir_lowering=False, num_devices=NCORES)

    xT = nc.declare_dram_parameter("xT", [D, ROWS], bf, isOutput=False)
    wq = nc.declare_dram_parameter("wq", [D, D], bf, isOutput=False)
    wk = nc.declare_dram_parameter("wk", [D, D], bf, isOutput=False)
    wv = nc.declare_dram_parameter("wv", [D, D], bf, isOutput=False)
    wo = nc.declare_dram_parameter("wo", [D, D], bf, isOutput=False)
    cosT = nc.declare_dram_parameter("cosT", [HD // 2, SQ], f32, isOutput=False)
    sinT = nc.declare_dram_parameter("sinT", [HD // 2, SQ], f32, isOutput=False)
    maskT = nc.declare_dram_parameter("maskT", [S, SQ], f32, isOutput=False)
    onesd = nc.declare_dram_parameter("ones", [128, 128], bf, isOutput=False)
    out = nc.declare_dram_parameter("out", [ROWS, D], f32, isOutput=True)

    with nc.allow_low_precision(reason="bf16 matmul inputs; fp32 accumulate"), \
         tile.TileContext(nc) as tc:
        with (
            tc.tile_pool(name="dram", bufs=1, space="DRAM") as dramp,
            tc.tile_pool(name="const", bufs=1) as constp,
            tc.tile_pool(name="wstream", bufs=10) as wpool,
            tc.tile_pool(name="sbout", bufs=4) as sbout,
        ):
            k_in = dramp.tile([D, ROWS], bf)
            k_outs = [
                dramp.tile([NCORES * 512, ROWS], bf, addr_space="Shared",
                           name=f"k_out{i}")
                for i in range(4)
            ]
            v_in = dramp.tile([ROWS, D], bf)
            v_out = dramp.tile([NCORES * ROWS, D], bf, addr_space="Shared")

            # resident tensors
            xts = constp.tile([128, DCH * ROWS], bf)         # x^T chunks
            for d in range(DCH):
                nc.sync.dma_start(
                    out=xts[:, d * ROWS:(d + 1) * ROWS],
                    in_=xT[d * 128:(d + 1) * 128, :],
                )
            cos_sb = constp.tile([64, SQ], f32)
            sin_sb = constp.tile([64, SQ], f32)
            nc.sync.dma_start(out=cos_sb[:], in_=cosT[:, :])
            nc.sync.dma_start(out=sin_sb[:], in_=sinT[:, :])
            mask_sb = constp.tile([128, 16 * SQ], f32)       # maskT chunks
            ones_sq = constp.tile([128, 128], bf)
            nc.sync.dma_start(out=ones_sq[:], in_=onesd[:, :])

            qsb = constp.tile([128, H * ROWS], bf)           # rope'd Q^T per head
            attn = constp.tile([128, H * ROWS], bf)          # attention out^T per head

            def rope(dst, dst_cols, src_ps, tmp_pool):
                # src_ps: [128, ROWS] psum, rows 0:64 = even dims, 64:128 = odd
                for b2 in range(B):
                    cs = slice(b2 * SQ, (b2 + 1) * SQ)
                    dcs = slice(dst_cols + b2 * SQ, dst_cols + (b2 + 1) * SQ)
                    te = src_ps[0:64, cs]
                    to = src_ps[64:128, cs]
                    t1 = tmp_pool.tile([64, SQ], f32, tag="ropetmp1")
                    t2 = tmp_pool.tile([64, SQ], f32, tag="ropetmp2")
                    nc.vector.tensor_mul(t1[:], te, cos_sb[:])
                    nc.vector.tensor_mul(t2[:], to, sin_sb[:])
                    nc.vector.tensor_sub(dst[0:64, dcs], t1[:], t2[:])
                    t3 = tmp_pool.tile([64, SQ], f32, tag="ropetmp3")
                    t4 = tmp_pool.tile([64, SQ], f32, tag="ropetmp4")
                    nc.vector.tensor_mul(t3[:], te, sin_sb[:])
                    nc.vector.tensor_mul(t4[:], to, cos_sb[:])
                    nc.vector.tensor_add(dst[64:128, dcs], t3[:], t4[:])

            # ---- K projection + RoPE -> k_in, AllGather ----
            with (
                tc.tile_pool(name="projps", bufs=8, space="PSUM") as projps,
                tc.tile_pool(name="ropetmp", bufs=3) as ropep,
            ):
                # ---- V projection (natural layout) -> v_in, AllGather ----
                for ncol in range(4):
                    vps = [projps.tile([128, 512], f32, tag="projps", name="projtile")
                           for _ in range(4)]
                    for d in range(DCH):
                        wvt = wpool.tile([128, 512], bf, tag="wst")
                        nc.sync.dma_start(
                            out=wvt[:],
                            in_=wv[d * 128:(d + 1) * 128, ncol * 512:(ncol + 1) * 512],
                        )
                        for rr in range(4):
                            nc.tensor.matmul(
                                vps[rr][:],
                                lhsT=xts[:, d * ROWS + rr * 128:d * ROWS + (rr + 1) * 128],
                                rhs=wvt[:],
                                start=(d == 0), stop=(d == DCH - 1),
                            )
                    for rr in range(4):
                        vsb = sbout.tile([128, 512], bf, tag="vsb")
                        nc.scalar.copy(vsb[:], vps[rr][:])
                        nc.scalar.dma_start(
                            out=v_in[rr * 128:(rr + 1) * 128, ncol * 512:(ncol + 1) * 512],
                            in_=vsb[:],
                        )
                nc.gpsimd.collective_compute(
                    "AllGather",
                    mybir.AluOpType.bypass,
                    replica_groups=[list(range(NCORES))],
                    ins=[v_in.opt()],
                    outs=[v_out.opt()],
                )

                # ---- K projection + RoPE -> k_in, AllGather ----
                for hg in range(4):          # head groups of 4
                    kps = [projps.tile([128, ROWS], f32, tag="projps", name="projtile")
                           for _ in range(4)]
                    for d in range(DCH):
                        wkt = wpool.tile([128, 512], bf, tag="wst")
                        nc.sync.dma_start(
                            out=wkt[:],
                            in_=wk[d * 128:(d + 1) * 128, hg * 512:(hg + 1) * 512],
                        )
                        for hh in range(4):
                            nc.tensor.matmul(
                                kps[hh][:],
                                lhsT=wkt[:, hh * 128:(hh + 1) * 128],
                                rhs=xts[:, d * ROWS:(d + 1) * ROWS],
                                start=(d == 0), stop=(d == DCH - 1),
                            )
                    for hh in range(4):
                        h = hg * 4 + hh
                        ksb = sbout.tile([128, ROWS], bf, tag="ksb")
                        rope(ksb, 0, kps[hh], ropep)
                        nc.scalar.dma_start(
                            out=k_in[h * 128:(h + 1) * 128, :], in_=ksb[:]
                        )
                    nc.gpsimd.collective_compute(
                        "AllGather",
                        mybir.AluOpType.bypass,
                        replica_groups=[list(range(NCORES))],
                        ins=[k_in[hg * 512:(hg + 1) * 512, :].opt()],
                        outs=[k_outs[hg].opt()],
                    )

                # ---- Q projection + RoPE (stays in SBUF) ----
                for hg in range(4):
                    qps = [projps.tile([128, ROWS], f32, tag="projps", name="projtile")
                           for _ in range(4)]
                    for d in range(DCH):
                        wqt = wpool.tile([128, 512], bf, tag="wst")
                        nc.sync.dma_start(
                            out=wqt[:],
                            in_=wq[d * 128:(d + 1) * 128, hg * 512:(hg + 1) * 512],
                        )
                        for hh in range(4):
                            nc.tensor.matmul(
                                qps[hh][:],
                                lhsT=wqt[:, hh * 128:(hh + 1) * 128],
                                rhs=xts[:, d * ROWS:(d + 1) * ROWS],
                                start=(d == 0), stop=(d == DCH - 1),
                            )
                    for hh in range(4):
                        h = hg * 4 + hh
                        rope(qsb, h * ROWS, qps[hh], ropep)

            # rank-major views of the gathered K^T and V for one-DMA-per-head
            # loads: K rows = rk*2048 + h*128 + d, V rows = cc*128 + p.
            k_views = [
                k_outs[i][:].rearrange(
                    "(rk h d) c -> h d rk c", rk=NCORES, h=4, d=128
                )
                for i in range(4)
            ]
            v_view = v_out[:].rearrange(
                "(cc p) (h t) -> h p cc t", cc=4 * NCORES, p=128, h=H, t=128
            )

            for kcb in range(16):
                nc.gpsimd.dma_start(
                    out=mask_sb[:, kcb * SQ:(kcb + 1) * SQ],
                    in_=maskT[kcb * 128:(kcb + 1) * 128, :],
                )

            # ---- Attention per head ----
            with (
                tc.tile_pool(name="ktiles", bufs=4) as kpool,
                tc.tile_pool(name="vtiles", bufs=4) as vpool,
                tc.tile_pool(name="scps", bufs=2, space="PSUM") as scps,
                tc.tile_pool(name="attps", bufs=4, space="PSUM") as attps,
                tc.tile_pool(name="smallps", bufs=1, space="PSUM") as smallps,
                tc.tile_pool(name="extiles", bufs=8) as expool,
                tc.tile_pool(name="tmp", bufs=3) as tmpp,
            ):
                for h in range(H):
                    kta = kpool.tile([128, NCORES * ROWS], bf, tag="kt")
                    nc.gpsimd.dma_start(out=kta[:], in_=k_views[h // 4][h % 4])
                    vta = vpool.tile([128, NCORES * ROWS], bf, tag="vt")
                    nc.gpsimd.dma_start(out=vta[:], in_=v_view[h])
                    att_ps = [
                        attps.tile([128, SQ], f32, tag="attps", name="atttile")
                        for _ in range(B)
                    ]
                    den = smallps.tile([1, ROWS], f32, tag="den")
                    for kc in range(16):
                        rk, koff = kc // 2, (kc % 2) * 128
                        sc = scps.tile([128, ROWS], f32, tag="scps")
                        for b2 in range(B):
                            nc.tensor.matmul(
                                sc[:, b2 * SQ:(b2 + 1) * SQ],
                                lhsT=kta[:, rk * ROWS + b2 * SQ + koff:
                                         rk * ROWS + b2 * SQ + koff + 128],
                                rhs=qsb[:, h * ROWS + b2 * SQ:h * ROWS + (b2 + 1) * SQ],
                                start=True, stop=True,
                            )
                        ex = expool.tile([128, ROWS], bf, tag="ex")
                        tmp = tmpp.tile([128, ROWS], f32, tag="tmp")
                        for b2 in range(B):
                            cs = slice(b2 * SQ, (b2 + 1) * SQ)
                            nc.vector.tensor_add(
                                tmp[:, cs], sc[:, cs],
                                mask_sb[:, kc * SQ:(kc + 1) * SQ],
                            )
                        nc.scalar.activation(ex[:], tmp[:], Exp, scale=SCALE)
                        # denominator: accumulate column sums of exp on the PE
                        nc.tensor.matmul(
                            den[:], lhsT=ones_sq[:, 0:1], rhs=ex[:],
                            start=(kc == 0), stop=(kc == 15),
                        )
                        for b2 in range(B):
                            ccb = rk * 4 + b2 * 2 + koff // 128
                            nc.tensor.matmul(
                                att_ps[b2][:],
                                lhsT=vta[:, ccb * 128:(ccb + 1) * 128],
                                rhs=ex[:, b2 * SQ:(b2 + 1) * SQ],
                                start=(kc == 0), stop=(kc == 15),
                            )
                    dsb = tmpp.tile([1, ROWS], bf, tag="dsb")
                    nc.scalar.copy(dsb[:], den[:])
                    rb = smallps.tile([128, ROWS], f32, tag="rb")
                    nc.tensor.matmul(
                        rb[:], lhsT=ones_sq[0:1, :], rhs=dsb[:],
                        start=True, stop=True,
                    )
                    rbs = tmpp.tile([128, ROWS], f32, tag="rbs")
                    rscr = tmpp.tile([128, ROWS], f32, tag="rscr")
                    nc.vector.reciprocal_approx_accurate(rbs[:], rb[:], rscr[:])
                    for b2 in range(B):
                        nc.vector.tensor_mul(
                            attn[:, h * ROWS + b2 * SQ:h * ROWS + (b2 + 1) * SQ],
                            att_ps[b2][:],
                            rbs[:, b2 * SQ:(b2 + 1) * SQ],
                        )

            # ---- Output projection ----
            with tc.tile_pool(name="ops", bufs=8, space="PSUM") as opsp:
                for nn in range(4):
                    ops = [opsp.tile([128, 512], f32, tag="ops", name="opstile")
                           for _ in range(4)]
                    for h in range(H):
                        wot = wpool.tile([128, 512], bf, tag="wst")
                        nc.sync.dma_start(
                            out=wot[:],
                            in_=wo[h * 128:(h + 1) * 128, nn * 512:(nn + 1) * 512],
                        )
                        for qt in range(4):
                            nc.tensor.matmul(
                                ops[qt][:],
                                lhsT=attn[:, h * ROWS + qt * 128:h * ROWS + (qt + 1) * 128],
                                rhs=wot[:],
                                start=(h == 0), stop=(h == H - 1),
                            )
                    for qt in range(4):
                        osb = sbout.tile([128, 512], f32, tag="osb")
                        nc.scalar.copy(osb[:], ops[qt][:])
                        nc.sync.dma_start(
                            out=out[qt * 128:(qt + 1) * 128, nn * 512:(nn + 1) * 512],
                            in_=osb[:],
                        )

    nc.compile()
    return nc


def _get_graph():
    global _GRAPH
    if _GRAPH is None:
        _GRAPH = _build_graph()
    return _GRAPH


_PERM = np.concatenate(
    [h * HD + np.concatenate([np.arange(0, HD, 2), np.arange(1, HD, 2)])
     for h in range(H)]
)


def kernel(x, Wq, Wk, Wv, Wo, freqs_cos, freqs_sin, mask):
    global _LAST_EXEC_NS, _LAST_RES
    from concourse.bass_utils import run_bass_kernel_spmd

    nc = _get_graph()

    x = np.asarray(x, np.float32)
    wq_p = np.ascontiguousarray(np.asarray(Wq, np.float32)[:, _PERM]).astype(BF16)
    wk_p = np.ascontiguousarray(np.asarray(Wk, np.float32)[:, _PERM]).astype(BF16)
    wv_b = np.ascontiguousarray(np.asarray(Wv, np.float32)).astype(BF16)
    wo_b = np.ascontiguousarray(np.asarray(Wo, np.float32)).astype(BF16)
    cosf = np.asarray(freqs_cos, np.float32)
    sinf = np.asarray(freqs_sin, np.float32)
    maskf = np.asarray(mask, np.float32)[0, 0]      # [S, S] (q, k)
    ones_b = np.ones((128, 128), BF16)

    in_maps = []
    for c in range(NCORES):
        rows = slice(c * SQ, (c + 1) * SQ)
        x_c = x[:, rows, :].reshape(ROWS, D)
        in_maps.append({
            "xT": np.ascontiguousarray(x_c.T).astype(BF16),
            "wq": wq_p, "wk": wk_p, "wv": wv_b, "wo": wo_b,
            "cosT": np.ascontiguousarray(cosf[rows].T),
            "sinT": np.ascontiguousarray(sinf[rows].T),
            "maskT": np.ascontiguousarray(maskf[rows].T * float(np.sqrt(HD))),
            "ones": ones_b,
        })

    res = run_bass_kernel_spmd(
        nc, in_maps, core_ids=list(range(NCORES)), trace=_TRACE,
    )
    _LAST_EXEC_NS = res.exec_time_ns
    _LAST_RES = res

    outp = np.empty((B, S, D), np.float32)
    for c in range(NCORES):
        o = res.results[c]["out"]
        for b in range(B):
            outp[b, c * SQ:(c + 1) * SQ, :] = o[b * SQ:(b + 1) * SQ, :]
    return outp



# revision 16
# speedup vs baseline: 1.3025x; 1.3025x over previous
"""Distributed Bass kernel for causal multi-head attention with RoPE.

Problem: B=2, S=2048, D=2048, H=16, HD=128 (nn_Attention_85315230368481).

Sharding: tensor-parallel over heads. Core c owns heads {2c, 2c+1} and
computes Q/K/V projections for those heads over the FULL sequence
(4096 rows = both batches), applies RoPE, then causal attention for its
2 heads (skipping fully-masked 128x512 key/query blocks), and finally
the output projection for its 512-row slice of the output. The per-head
attention outputs are exchanged with a single AllToAll per head (each
core sends its heads' columns split by destination row-slice and
receives every head's values for its own rows) -- 8x less fabric
traffic than an AllGather of K/V or of attention outputs.

Layout tricks:
 - x arrives transposed ([D, 4096]); Q^T/K^T come out of the PE as
   [head_dim, rows] and V in natural [rows, head_dim], so no on-chip
   transposes are needed anywhere.
 - Wq/Wk columns are permuted per head (even dims then odd dims) so
   RoPE works on contiguous partition halves; scores are invariant to
   the permutation since both Q and K use it.
 - Scores are computed transposed ([keys, queries]); softmax
   denominators accumulate on the vector engine (one add per exp tile)
   with a single ones-matmul partition-reduction per query tile.
 - exp is computed shifted (exp(s/sqrt(hd) - 5)) so probabilities and
   denominators stay in fp16 range; softmax is shift-invariant.
 - Diagonal 128x512 blocks use one of 4 precomputed relative causal
   masks; fully-masked blocks are skipped outright.
 - Wo rows are pre-permuted to AllToAll arrival order so the output
   projection consumes exchange chunks directly.
 - All matmuls and element-wise ops in fp16 (fp32 accumulation in
   PSUM; fp32 softmax denominator reduction) -- full PE rate and 2-4x
   DVE rate vs fp32.
"""

import sys

import numpy as np

if "/opt/trn_rl_repo" not in sys.path:
    sys.path.insert(0, "/opt/trn_rl_repo")

B, S, D, H = 2, 2048, 2048, 16
HD = D // H            # 128
NCORES = 8
HPC = H // NCORES      # 2 heads per core
ROWS = B * S           # 4096 rows total (both batches)
ORON = ROWS // NCORES  # 512 output rows per core
DCH = D // 128         # 16 contraction chunks
NQT = S // 512         # 4 query tiles of 512 per batch
NRT = ROWS // 512      # 8 row tiles of 512 (projection)
SCALE = 1.0 / float(np.sqrt(HD))
ESHIFT = -5.0          # exp(s*SCALE + ESHIFT): keeps sums in fp16 range
F16 = np.float16

_GRAPH = None
_TRACE = False
_LAST_EXEC_NS = None
_LAST_RES = None


def _build_graph():
    import concourse.mybir as mybir
    from concourse import bacc, tile

    f32 = mybir.dt.float32
    f16 = mybir.dt.float16
    Exp = mybir.ActivationFunctionType.Exp

    nc = bacc.Bacc("TRN2", target_bir_lowering=False, num_devices=NCORES)

    xT = nc.declare_dram_parameter("xT", [D, ROWS], f16, isOutput=False)
    wq = nc.declare_dram_parameter("wq", [D, HPC * HD], f16, isOutput=False)
    wk = nc.declare_dram_parameter("wk", [D, HPC * HD], f16, isOutput=False)
    wv = nc.declare_dram_parameter("wv", [D, HPC * HD], f16, isOutput=False)
    wo = nc.declare_dram_parameter("wo", [D, D], f16, isOutput=False)
    # cos/sin duplicated across both partition halves (DVE same-base rule)
    cosT = nc.declare_dram_parameter("cosT", [HD, S], f16, isOutput=False)
    sinT = nc.declare_dram_parameter("sinT", [HD, S], f16, isOutput=False)
    # 4 relative diagonal-mask blocks [128, 512], pre-scaled by sqrt(HD)
    maskd = nc.declare_dram_parameter("maskd", [128, 4 * 512], f32, isOutput=False)
    onesh = nc.declare_dram_parameter("onesh", [128, 128], f16, isOutput=False)
    out = nc.declare_dram_parameter("out", [ORON, D], f32, isOutput=True)

    with nc.allow_low_precision(reason="fp16 matmul/vector; fp32 accumulate"), \
         tile.TileContext(nc) as tc:
        with (
            tc.tile_pool(name="dram", bufs=1, space="DRAM") as dramp,
            tc.tile_pool(name="resid", bufs=1) as resid,
        ):
            a2a_in = [dramp.tile([NCORES * HD, ORON], f16, name=f"a2ain{h}")
                      for h in range(HPC)]
            a2a_out = [dramp.tile([NCORES * HD, ORON], f16, name=f"a2aout{h}")
                       for h in range(HPC)]

            # long-lived SBUF tensors
            cos_sb = resid.tile([128, S], f16)
            sin_sb = resid.tile([128, S], f16)
            nc.sync.dma_start(out=cos_sb[:], in_=cosT[:, :])
            nc.sync.dma_start(out=sin_sb[:], in_=sinT[:, :])
            mask_sb = resid.tile([128, 4 * 512], f32)
            nc.sync.dma_start(out=mask_sb[:], in_=maskd[:, :])
            ones_sb = resid.tile([128, 128], f16)
            nc.sync.dma_start(out=ones_sb[:], in_=onesh[:, :])

            eshift_sb = resid.tile([128, 1], f32)
            nc.vector.memset(eshift_sb[:], ESHIFT)

            qT = resid.tile([128, HPC * ROWS], f16)   # rope'd Q^T per head
            kT = resid.tile([128, HPC * ROWS], f16)   # rope'd K^T per head
            v_sb = resid.tile([128, (ROWS // 128) * HPC * HD], f16)
            attT = resid.tile([128, HPC * ROWS], f16)  # attention out^T per head

            # ---- Q/K/V projections (+ RoPE on Q,K) ----
            with (
                tc.tile_pool(name="wsb", bufs=1) as wsb,
                tc.tile_pool(name="xstream", bufs=4) as xpool,
                tc.tile_pool(name="qkps", bufs=1, space="PSUM") as qkps,
                tc.tile_pool(name="vps", bufs=1, space="PSUM") as vpsp,
                tc.tile_pool(name="drains", bufs=2) as drainp,
                tc.tile_pool(name="ropetmp", bufs=2) as ropep,
            ):
                wq_sb = wsb.tile([128, DCH * HPC * HD], f16)
                wk_sb = wsb.tile([128, DCH * HPC * HD], f16)
                wv_sb = wsb.tile([128, DCH * HPC * HD], f16)
                for t_sb, t_dram in ((wq_sb, wq), (wk_sb, wk), (wv_sb, wv)):
                    for d in range(DCH):
                        nc.sync.dma_start(
                            out=t_sb[:, d * HPC * HD:(d + 1) * HPC * HD],
                            in_=t_dram[d * 128:(d + 1) * 128, :],
                        )

                def rope(dst, dst_col, src, s0):
                    # src: [128, 512] sbuf fp16; rows 0:64 even dims, 64:128 odd
                    te = src[0:64, :]
                    to = src[64:128, :]
                    cl = cos_sb[0:64, s0:s0 + 512]
                    ch = cos_sb[64:128, s0:s0 + 512]
                    sl = sin_sb[0:64, s0:s0 + 512]
                    sh = sin_sb[64:128, s0:s0 + 512]
                    t1 = ropep.tile([64, 512], f16, tag="r1")
                    t2 = ropep.tile([64, 512], f16, tag="r2")
                    nc.vector.tensor_mul(t1[:], te, cl)
                    nc.vector.tensor_mul(t2[:], to, sh)
                    nc.vector.tensor_sub(dst[0:64, dst_col:dst_col + 512],
                                         t1[:], t2[:])
                    t3 = ropep.tile([64, 512], f16, tag="r3")
                    t4 = ropep.tile([64, 512], f16, tag="r4")
                    nc.vector.tensor_mul(t3[:], te, sl)
                    nc.vector.tensor_mul(t4[:], to, ch)
                    nc.vector.tensor_add(dst[64:128, dst_col:dst_col + 512],
                                         t3[:], t4[:])

                for rt in range(NRT):
                    s0 = (rt % NQT) * 512
                    qk = [qkps.tile([128, 512], f32, tag=f"qk{i}",
                                    name="qktile") for i in range(4)]
                    vp = [vpsp.tile([128, 256], f32, tag=f"vp{i}",
                                    name="vptile") for i in range(4)]
                    for d in range(DCH):
                        x_t = xpool.tile([128, 512], f16, tag="xt")
                        nc.sync.dma_start(
                            out=x_t[:],
                            in_=xT[d * 128:(d + 1) * 128,
                                   rt * 512:(rt + 1) * 512],
                        )
                        first, last = d == 0, d == DCH - 1
                        c0 = d * HPC * HD
                        for hh in range(HPC):
                            nc.tensor.matmul(
                                qk[hh][:],
                                lhsT=wq_sb[:, c0 + hh * HD:c0 + (hh + 1) * HD],
                                rhs=x_t[:], start=first, stop=last,
                            )
                            nc.tensor.matmul(
                                qk[2 + hh][:],
                                lhsT=wk_sb[:, c0 + hh * HD:c0 + (hh + 1) * HD],
                                rhs=x_t[:], start=first, stop=last,
                            )
                        for sub in range(4):
                            nc.tensor.matmul(
                                vp[sub][:],
                                lhsT=x_t[:, sub * 128:(sub + 1) * 128],
                                rhs=wv_sb[:, c0:c0 + HPC * HD],
                                start=first, stop=last,
                            )
                    # fast drain: psum -> sbuf fp16 copies, then RoPE off-psum
                    for i in range(4):
                        qksb = drainp.tile([128, 512], f16, tag=f"dr{i}")
                        nc.scalar.copy(qksb[:], qk[i][:])
                        dst = qT if i < 2 else kT
                        rope(dst, (i % 2) * ROWS + rt * 512, qksb, s0)
                    for sub in range(4):
                        gc = rt * 4 + sub
                        nc.scalar.copy(
                            v_sb[:, gc * 256:(gc + 1) * 256], vp[sub][:])

            # ---- Attention per (head, batch), causal-skipped ----
            with tc.tile_pool(name="wop", bufs=1) as wop:
                # Wo resident, chunk-major (prefetched during attention)
                wo_sb = wop.tile([128, DCH * D], f16)
                for k in range(DCH):
                    nc.sync.dma_start(
                        out=wo_sb[:, k * D:(k + 1) * D],
                        in_=wo[k * 128:(k + 1) * 128, :],
                    )
                with (
                    tc.tile_pool(name="scps", bufs=4, space="PSUM") as scps,
                    tc.tile_pool(name="attps", bufs=1, space="PSUM") as attps,
                    tc.tile_pool(name="extiles", bufs=6) as expool,
                    tc.tile_pool(name="esum", bufs=2) as esump,
                    tc.tile_pool(name="tmp", bufs=2) as tmpp,
                ):
                    for h in range(HPC):
                        for b in range(B):
                            kcol = h * ROWS + b * S
                            att = [attps.tile([128, 512], f32, tag=f"att{qt}",
                                              name="atttile")
                                   for qt in range(NQT)]
                            esum = [esump.tile([128, 512], f16, tag=f"es{qt}",
                                               name="esumtile")
                                    for qt in range(NQT)]
                            pend = []

                            def flush_attv(limit):
                                while len(pend) > limit:
                                    qt2, kc2, ex2 = pend.pop(0)
                                    gc2 = (b * DCH + kc2) * HPC * HD + h * HD
                                    nc.tensor.matmul(
                                        att[qt2][:],
                                        lhsT=v_sb[:, gc2:gc2 + HD],
                                        rhs=ex2[:],
                                        start=(kc2 == 0),
                                        stop=(kc2 == 4 * qt2 + 3),
                                    )

                            for kc in range(DCH):
                                for qt in range(kc // 4, NQT):
                                    sc = scps.tile([128, 512], f32, tag="sc")
                                    nc.tensor.matmul(
                                        sc[:],
                                        lhsT=kT[:, kcol + kc * 128:
                                                kcol + (kc + 1) * 128],
                                        rhs=qT[:, kcol + qt * 512:
                                               kcol + (qt + 1) * 512],
                                        start=True, stop=True,
                                    )
                                    if kc // 4 == qt:  # diagonal: apply mask
                                        kk = kc % 4
                                        nc.vector.tensor_add(
                                            sc[:], sc[:],
                                            mask_sb[:, kk * 512:(kk + 1) * 512],
                                        )
                                    ex = expool.tile([128, 512], f16, tag="ex")
                                    nc.scalar.activation(ex[:], sc[:], Exp,
                                                         bias=eshift_sb[:],
                                                         scale=SCALE)
                                    if kc == 0:
                                        nc.vector.tensor_copy(esum[qt][:], ex[:])
                                    else:
                                        nc.vector.tensor_add(
                                            esum[qt][:], esum[qt][:], ex[:])
                                    pend.append((qt, kc, ex))
                                    flush_attv(3)
                            flush_attv(0)
                            # normalize: den = ones^T @ esum; bcast 1/den
                            for qt in range(NQT):
                                den = scps.tile([128, 512], f32, tag="sc")
                                nc.tensor.matmul(
                                    den[0:1, :], lhsT=ones_sb[:, 0:1],
                                    rhs=esum[qt][:], start=True, stop=True,
                                )
                                rcp = tmpp.tile([1, 512], f16, tag="rcp")
                                nc.vector.reciprocal(rcp[:], den[0:1, :])
                                rb = scps.tile([128, 512], f32, tag="sc")
                                nc.tensor.matmul(
                                    rb[:], lhsT=ones_sb[0:1, :], rhs=rcp[:],
                                    start=True, stop=True,
                                )
                                rbs = tmpp.tile([128, 512], f16, tag="rbs")
                                nc.vector.tensor_copy(rbs[:], rb[:])
                                nc.vector.tensor_mul(
                                    attT[:, kcol + qt * 512:
                                         kcol + (qt + 1) * 512],
                                    att[qt][:], rbs[:],
                                )
                        # exchange this head's attention output
                        for dd in range(NCORES):
                            nc.sync.dma_start(
                                out=a2a_in[h][dd * 128:(dd + 1) * 128, :],
                                in_=attT[:, h * ROWS + dd * 512:
                                         h * ROWS + (dd + 1) * 512],
                            )
                        nc.gpsimd.collective_compute(
                            "AllToAll",
                            mybir.AluOpType.bypass,
                            replica_groups=[list(range(NCORES))],
                            ins=[a2a_in[h][:].opt()],
                            outs=[a2a_out[h][:].opt()],
                        )

                # ---- Output projection over exchange chunks ----
                with (
                    tc.tile_pool(name="atile", bufs=1) as atp,
                    tc.tile_pool(name="ops", bufs=2, space="PSUM") as opsp,
                    tc.tile_pool(name="osb", bufs=2) as osbp,
                ):
                    aT = atp.tile([128, DCH * 512], f16)
                    for k in range(DCH):
                        h, r = k // NCORES, k % NCORES
                        nc.gpsimd.dma_start(
                            out=aT[:, k * 512:(k + 1) * 512],
                            in_=a2a_out[h][r * 128:(r + 1) * 128, :],
                        )
                    for m in range(4):
                        ops = opsp.tile([128, D], f32, tag="ops",
                                        name="opstile")
                        for k in range(DCH):
                            for n in range(4):
                                nc.tensor.matmul(
                                    ops[:, n * 512:(n + 1) * 512],
                                    lhsT=aT[:, k * 512 + m * 128:
                                            k * 512 + (m + 1) * 128],
                                    rhs=wo_sb[:, k * D + n * 512:
                                              k * D + (n + 1) * 512],
                                    start=(k == 0), stop=(k == DCH - 1),
                                )
                        osb = osbp.tile([128, D], f32, tag="osb")
                        nc.scalar.copy(osb[:], ops[:])
                        nc.sync.dma_start(
                            out=out[m * 128:(m + 1) * 128, :], in_=osb[:],
                        )

    nc.compile()
    return nc


def _get_graph():
    global _GRAPH
    if _GRAPH is None:
        _GRAPH = _build_graph()
    return _GRAPH


_EVENODD = np.concatenate([np.arange(0, HD, 2), np.arange(1, HD, 2)])


def kernel(x, Wq, Wk, Wv, Wo, freqs_cos, freqs_sin, mask):
    global _LAST_EXEC_NS, _LAST_RES
    from concourse.bass_utils import run_bass_kernel_spmd

    nc = _get_graph()

    x = np.asarray(x, np.float32)
    Wq = np.asarray(Wq, np.float32)
    Wk = np.asarray(Wk, np.float32)
    Wv = np.asarray(Wv, np.float32)
    Wo = np.asarray(Wo, np.float32)

    xTb = np.ascontiguousarray(x.reshape(ROWS, D).T).astype(F16)
    cosf = np.asarray(freqs_cos, np.float32).T.astype(F16)
    sinf = np.asarray(freqs_sin, np.float32).T.astype(F16)
    cosf = np.ascontiguousarray(np.concatenate([cosf, cosf], axis=0))
    sinf = np.ascontiguousarray(np.concatenate([sinf, sinf], axis=0))

    # 4 relative diagonal mask blocks, pre-scaled by sqrt(HD)
    i_idx = np.arange(128)[:, None]
    q_idx = np.arange(512)[None, :]
    maskd = np.concatenate(
        [np.where(kk * 128 + i_idx > q_idx, np.float32(-1e9 * np.sqrt(HD)),
                  np.float32(0.0)) for kk in range(4)], axis=1)
    maskd = np.ascontiguousarray(maskd, np.float32)

    # Wo rows permuted to AllToAll arrival order: heads 0,2,..,14,1,3,..,15
    row_order = np.concatenate(
        [np.arange(h * HD, (h + 1) * HD)
         for h in [2 * r for r in range(NCORES)]
         + [2 * r + 1 for r in range(NCORES)]])
    wo_p = np.ascontiguousarray(Wo[row_order, :]).astype(F16)
    ones_h = np.ones((128, 128), F16)

    in_maps = []
    for c in range(NCORES):
        cols_pq = np.concatenate(
            [(2 * c + hh) * HD + _EVENODD for hh in range(HPC)])
        cols_v = np.arange(2 * c * HD, (2 * c + HPC) * HD)
        in_maps.append({
            "xT": xTb,
            "wq": np.ascontiguousarray(Wq[:, cols_pq]).astype(F16),
            "wk": np.ascontiguousarray(Wk[:, cols_pq]).astype(F16),
            "wv": np.ascontiguousarray(Wv[:, cols_v]).astype(F16),
            "wo": wo_p,
            "cosT": cosf, "sinT": sinf,
            "maskd": maskd, "onesh": ones_h,
        })

    res = run_bass_kernel_spmd(
        nc, in_maps, core_ids=list(range(NCORES)), trace=_TRACE,
    )
    _LAST_EXEC_NS = res.exec_time_ns
    _LAST_RES = res

    outp = np.empty((ROWS, D), np.float32)
    for c in range(NCORES):
        outp[c * ORON:(c + 1) * ORON, :] = res.results[c]["out"]
    return outp.reshape(B, S, D)
